# revision 2
# baseline (speedup 1.0000x reference)
"""3-layer GCN (DGL GraphConv, norm='both') on 8 Trainium2 NeuronCores.

Strategy:
  - Nodes are packed into 80 balanced bins (128 slots each) by in-degree
    (greedy least-loaded), 10 bins per core -> 1280 padded rows/core.
  - Edges live with the owner (bin) of their dst node. segment_sum is done
    as "scatter matmuls" on the TensorEngine: for each dst block,
    agg[128d, D] += S_kt[128s, 128d].T @ msg_kt[128s, D], where msg rows are
    the DISTINCT src nodes of the block's edges (deduped), fetched with
    dma_gather (SWDGE), and S is a host-built sparse matrix carrying
    sum-of-edge-weights norm_src[src]*norm_dst[dst] at (src_slot, dst_slot).
  - The whole on-device datapath is fp16 (PSUM accumulation stays fp32):
    halves the gather HBM traffic, the AllGather bytes, and the S/W loads.
  - Dense W matmuls run per dst block: PE-transpose agg -> aggT, then
    x = aggT.T @ W (+ bias via K=1 matmul) with ReLU fused into the
    PSUM->SBUF copy.
  - Layer outputs are exchanged with staged ncfw AllGathers so every core
    can gather any source row for the next layer's SpMM.
  - Layer 3 computes y3 = x3 @ W3 locally first (64 wide, padded to 128 for
    the gather's 256B row-size floor), AllGathers the small y3, then
    aggregates: A (x W3) == (A x) W3.
"""
import sys
sys.path.insert(0, '/opt/trn_rl_repo')
import numpy as np

N_CORES = 8
CHMAX = 5        # k-tiles per gather chunk (layers 1/2)
CHMAX3 = 8       # k-tiles per gather chunk (layer 3)


def _ag_splits(nblk):
    """Block-index boundaries of the staged AllGather slabs."""
    if nblk <= 2:
        return [0, nblk]
    fr = [0, round(0.3 * nblk), round(0.6 * nblk), round(0.8 * nblk),
          nblk - 1, nblk]
    return sorted(set(b for b in fr if 0 <= b <= nblk))


def _chunks(kt, chmax):
    """Split kt k-tiles into near-equal chunks of size <= chmax."""
    n = -(-kt // chmax)
    base, rem = divmod(kt, n)
    sizes = [base + 1] * rem + [base] * (n - rem)
    out, o = [], 0
    for s in sizes:
        out.append((o, s))
        o += s
    return out


# ---------------------------------------------------------------- host prep
def _partition_nodes(deg_in, n_nodes, nbins):
    """Greedy balanced-edge binning: nodes (sorted by in-degree desc) go to
    the least-loaded bin with a free slot (capacity 128)."""
    import heapq
    order = np.argsort(-deg_in, kind="stable")
    heap = [(0, b) for b in range(nbins)]
    heapq.heapify(heap)
    bin_of = np.empty(n_nodes, np.int32)
    slot_of = np.empty(n_nodes, np.int32)
    count = np.zeros(nbins, np.int64)
    load = np.zeros(nbins, np.int64)
    for n in order:
        while True:
            l, b = heapq.heappop(heap)
            if count[b] < 128:
                break
            # full bin: drop from heap permanently
        bin_of[n] = b
        slot_of[n] = count[b]
        count[b] += 1
        load[b] += int(deg_in[n])
        heapq.heappush(heap, (l + int(deg_in[n]), b))
    return bin_of, slot_of, load


def _prep(h, src, dst, cfg):
    """Build per-core S tiles, gather indices, and row maps."""
    N, E, NBLK = cfg["N"], cfg["E"], cfg["NBLK"]
    nbins = N_CORES * NBLK
    deg_out = np.bincount(src, minlength=N)
    deg_in = np.bincount(dst, minlength=N)
    norm_src = np.clip(deg_out, 1, None).astype(np.float32) ** np.float32(-0.5)
    norm_dst = np.clip(deg_in, 1, None).astype(np.float32) ** np.float32(-0.5)
    w = (norm_src[src] * norm_dst[dst]).astype(np.float32)

    bin_of, slot_of, load = _partition_nodes(deg_in, N, nbins)

    # deal bins to cores snake-wise by load to balance core totals
    order = np.argsort(-load, kind="stable")
    core_of_bin = np.empty(nbins, np.int32)
    blk_of_bin = np.empty(nbins, np.int32)
    nextblk = [0] * N_CORES
    for i, b in enumerate(order):
        r = i // N_CORES
        c = (i % N_CORES) if r % 2 == 0 else (N_CORES - 1 - (i % N_CORES))
        core_of_bin[b] = c
        blk_of_bin[b] = nextblk[c]
        nextblk[c] += 1

    RPC = NBLK * 128
    row_of_node = (core_of_bin[bin_of] * RPC + blk_of_bin[bin_of] * 128
                   + slot_of).astype(np.int32)
    # gather-id layout after the staged slab AllGathers: slab q holds rows
    # [b_q, e_q) of every core, concatenated core-major at offset 8*b_q
    sp = np.array(_ag_splits(NBLK)) * 128
    _c = row_of_node // RPC
    _r = row_of_node % RPC
    _q = np.searchsorted(sp, _r, side="right") - 1
    gid_of_node = (N_CORES * sp[_q] + _c * (sp[_q + 1] - sp[_q])
                   + _r - sp[_q]).astype(np.int32)

    # group edges by dst bin; dedup src nodes within each bin
    ebin = bin_of[dst]
    eorder = np.argsort(ebin, kind="stable")
    counts = np.bincount(ebin, minlength=nbins)
    bounds = np.concatenate([[0], np.cumsum(counts)])
    uniqs = []
    for b in range(nbins):
        es = eorder[bounds[b]:bounds[b + 1]]
        u, inv = np.unique(src[es], return_inverse=True)
        uniqs.append((es, u, inv))
    max_u = max(len(u) for _, u, _ in uniqs)
    kt_blk = max(cfg["KT_MIN"], -(-max_u // 128))
    kt_tot = NBLK * kt_blk

    idx1 = np.zeros((N_CORES, kt_tot * 128), np.int16)
    idx23 = np.zeros((N_CORES, kt_tot * 128), np.int16)
    S = np.zeros((N_CORES, 128, kt_tot, 128), np.float32)
    for b in range(nbins):
        es, u, inv = uniqs[b]
        c, blk = int(core_of_bin[b]), int(blk_of_bin[b])
        p = np.arange(len(u))
        gpos = blk * kt_blk * 128 + p
        idx1[c, gpos] = u.astype(np.int16)
        idx23[c, gpos] = gid_of_node[u].astype(np.int16)
        kt = blk * kt_blk + inv // 128
        np.add.at(S[c], (inv % 128, kt, slot_of[dst[es]]), w[es])
    S = S.astype(np.float16)

    def wrap(ix):  # -> [128, kt_tot*8] wrapped for the 8 Q7 cores
        return np.tile(ix.reshape(-1, 16).T, (8, 1)).copy()

    idx1_w = np.stack([wrap(idx1[c]) for c in range(N_CORES)])
    idx23_w = np.stack([wrap(idx23[c]) for c in range(N_CORES)])
    return dict(S=S, idx1=idx1_w, idx23=idx23_w, row_of_node=row_of_node,
                kt_blk=kt_blk, kt_tot=kt_tot)


# ---------------------------------------------------------------- device prog
def _build(cfg, kt_blk, use_bias):
    import concourse.bacc as bacc
    import concourse.mybir as mybir
    import concourse.tile as tile
    from concourse.library_config import mlp

    f32 = mybir.dt.float32
    f16 = mybir.dt.float16
    i16 = mybir.dt.int16
    RELU = mybir.ActivationFunctionType.Relu
    COPY = mybir.ActivationFunctionType.Copy

    N, D, C, NBLK = cfg["N"], cfg["D"], cfg["C"], cfg["NBLK"]
    RPC = NBLK * 128
    NPAD = N_CORES * RPC
    KT = kt_blk
    KT_TOT = NBLK * KT
    C3 = 128                    # y3 rows padded to 256B for the gather
    CHUNKS = _chunks(KT, CHMAX)
    CHUNKS3 = _chunks(KT, CHMAX3)
    KD = D // 128               # dense contraction k-tiles
    ND = 512 if D % 512 == 0 else D
    NT = D // ND                # dense n-tiles
    TPW = min(512, D)           # transposes packed per tps tile
    TPG = TPW // 128
    SPL = _ag_splits(NBLK)

    nc = bacc.Bacc("TRN2", target_bir_lowering=False, debug=False,
                   num_devices=N_CORES, num_swdge_queues=4,
                   dynamic_dma_scratch_size=32768)

    hx = nc.dram_tensor("hx", [N, D], f16, kind="ExternalInput")
    sker = nc.dram_tensor("sker", [128, KT_TOT, 128], f16, kind="ExternalInput")
    idx1_h = nc.dram_tensor("idx1", [128, KT_TOT * 8], i16, kind="ExternalInput")
    idx23_h = nc.dram_tensor("idx23", [128, KT_TOT * 8], i16, kind="ExternalInput")
    w12_h = nc.dram_tensor("w12", [2, 128, KD, D], f16, kind="ExternalInput")
    w3_h = nc.dram_tensor("w3", [128, KD, C], f16, kind="ExternalInput")
    ident_h = nc.dram_tensor("ident", [128, 128], f16, kind="ExternalInput")
    bias_h = nc.dram_tensor("biases", [1, 2 * D + C + 128], f16, kind="ExternalInput")
    out_h = nc.dram_tensor("out", [RPC, C], f32, kind="ExternalOutput")

    ag_in = nc.dram_tensor("ag_in", [RPC, D], f16, kind="Internal")
    ag_out = nc.dram_tensor("ag_out", [NPAD, D], f16, kind="Internal",
                            addr_space="Shared")
    ag3_in = nc.dram_tensor("ag3_in", [RPC, C3], f16, kind="Internal")
    ag3_out = nc.dram_tensor("ag3_out", [NPAD, C3], f16, kind="Internal",
                             addr_space="Shared")

    with tile.TileContext(nc) as tc:
        nc.gpsimd.load_library(mlp)
        with (
            tc.tile_pool(name="const", bufs=1) as cp,
            tc.tile_pool(name="msg", bufs=3) as mp,
            tc.tile_pool(name="msg3", bufs=2) as mp3,
            tc.tile_pool(name="work", bufs=2) as wp,
            tc.tile_pool(name="aggps", bufs=2, space="PSUM") as aps,
            tc.tile_pool(name="densps", bufs=2, space="PSUM") as dps,
            tc.tile_pool(name="tpsps", bufs=2, space="PSUM") as tps,
        ):
            idx1_t = cp.tile([128, KT_TOT * 8], i16, tag="idx1")
            nc.sync.dma_start(idx1_t[:], idx1_h[:])
            s_blk = []
            for b in range(NBLK):
                sb = cp.tile([128, KT, 128], f16, tag=f"s{b}")
                nc.sync.dma_start(sb[:], sker[:, b * KT:(b + 1) * KT, :])
                s_blk.append(sb)
                if b == 0:
                    w_t = cp.tile([128, KD, D], f16, tag="w")
                    nc.sync.dma_start(w_t[:], w12_h[0])
                    ident_t = cp.tile([128, 128], f16, tag="ident")
                    nc.sync.dma_start(ident_t[:], ident_h[:])
            idx23_t = cp.tile([128, KT_TOT * 8], i16, tag="idx23")
            nc.sync.dma_start(idx23_t[:], idx23_h[:])
            w3_t = cp.tile([128, KD, C], f16, tag="w3")
            nc.sync.dma_start(w3_t[:], w3_h[:])
            if use_bias:
                brow_t = cp.tile([1, 2 * D + C + 128], f16, tag="brow")
                nc.sync.dma_start(brow_t[:], bias_h[:])
                ones_t = brow_t[:, 2 * D + C:2 * D + C + 128]

            qctr = [0]

            def spmm_block(b, src_ap, idx_t, width, rwidth, chunk_list, chpool,
                           msg_pool, psum_pool, close=True):
                """agg[128, rwidth] for dst block b via gather + scatter MMs."""
                agg = psum_pool.tile([128, rwidth], f32, tag="aggps")
                nspl = max(1, rwidth // 512)
                for c0, ch in chunk_list:
                    msg = msg_pool.tile([128, chpool, width], f16, tag="m")
                    col0 = (b * KT + c0) * 8
                    q = qctr[0] % 4
                    qctr[0] += 1
                    nc.gpsimd.dma_gather(
                        msg[:, :ch, :], src_ap, idx_t[:, col0:col0 + ch * 8],
                        ch * 128, ch * 128, width, queue_num=q)
                    for k in range(ch):
                        kt = c0 + k
                        first = (c0 == 0 and k == 0)
                        last = (c0 + ch == KT and k == ch - 1)
                        for n in range(nspl):
                            w0 = n * (rwidth // nspl)
                            w1 = (n + 1) * (rwidth // nspl)
                            nc.tensor.matmul(
                                agg[:, w0:w1], s_blk[b][:, kt, :],
                                msg[:, k, w0:w1],
                                start=first, stop=last and close)
                return agg

            def transpose_to(dst_t, src_sb):
                """dst_t[128, KD, 128] (f16) = src_sb[128, D] transposed."""
                for g in range(KD // TPG):
                    tp = tps.tile([128, TPW], f16, tag="tp")
                    for j in range(TPG):
                        col = (g * TPG + j) * 128
                        nc.tensor.transpose(
                            tp[:, j * 128:(j + 1) * 128],
                            src_sb[:, col:col + 128], ident_t[:])
                    nc.vector.tensor_copy(
                        dst_t[:, g * TPG:(g + 1) * TPG, :].rearrange(
                            "p a b -> p (a b)"), tp[:])

            def dense_block(aggT_t, out_sb, bias_off, relu):
                """out_sb[128, D] = act(aggT.T @ W + b)."""
                for n in range(NT):
                    dp = dps.tile([128, ND], f32, tag="dp")
                    for k in range(KD):
                        nc.tensor.matmul(
                            dp[:], aggT_t[:, k, :], w_t[:, k, n * ND:(n + 1) * ND],
                            start=(k == 0), stop=(k == KD - 1 and not use_bias))
                    if use_bias:
                        nc.tensor.matmul(
                            dp[:], ones_t,
                            brow_t[:, bias_off + n * ND:bias_off + (n + 1) * ND],
                            start=False, stop=True)
                    nc.scalar.activation(out_sb[:, n * ND:(n + 1) * ND], dp[:],
                                         RELU if relu else COPY)

            # ---------------- layer 1 + 2
            for layer in range(2):
                src_ap = hx[:] if layer == 0 else ag_out[:]
                idx_t = idx1_t if layer == 0 else idx23_t
                for b in range(NBLK):
                    agg = spmm_block(b, src_ap, idx_t, D, D, CHUNKS, CHMAX,
                                     mp, aps)
                    agg_sb = wp.tile([128, D], f16, tag="aggsb")
                    nc.scalar.activation(agg_sb[:], agg[:], COPY)
                    aggT_t = wp.tile([128, KD, 128], f16, tag="aggT")
                    transpose_to(aggT_t, agg_sb)
                    x_sb = wp.tile([128, D], f16, tag="x")
                    dense_block(aggT_t, x_sb, layer * D, relu=True)
                    if layer == 0:
                        nc.sync.dma_start(ag_in[b * 128:(b + 1) * 128, :], x_sb[:])
                        if b + 1 in SPL[1:]:
                            r0, r1 = SPL[SPL.index(b + 1) - 1] * 128, (b + 1) * 128
                            nc.gpsimd.collective_compute(
                                "AllGather", mybir.AluOpType.bypass,
                                ins=[ag_in[r0:r1, :]],
                                outs=[ag_out[N_CORES * r0:N_CORES * r1, :]],
                                replica_groups=[list(range(N_CORES))])
                    else:
                        # y3 = x3 @ W3 for this block
                        x3T_t = wp.tile([128, KD, 128], f16, tag="x3T")
                        transpose_to(x3T_t, x_sb)
                        yp = dps.tile([128, C], f32, tag="dp")
                        for k in range(KD):
                            nc.tensor.matmul(yp[:], x3T_t[:, k, :], w3_t[:, k, :],
                                             start=(k == 0), stop=(k == KD - 1))
                        y_sb = wp.tile([128, C3], f16, tag="y")
                        nc.scalar.activation(y_sb[:, :C], yp[:], COPY)
                        nc.sync.dma_start(ag3_in[b * 128:(b + 1) * 128, :], y_sb[:])
                        if b + 1 in SPL[1:]:
                            r0, r1 = SPL[SPL.index(b + 1) - 1] * 128, (b + 1) * 128
                            nc.gpsimd.collective_compute(
                                "AllGather", mybir.AluOpType.bypass,
                                ins=[ag3_in[r0:r1, :]],
                                outs=[ag3_out[N_CORES * r0:N_CORES * r1, :]],
                                replica_groups=[list(range(N_CORES))])
                if layer == 0:
                    nc.sync.dma_start(w_t[:], w12_h[1])

            # ---------------- layer 3: out = A y3 (+ b3)
            for b in range(NBLK):
                agg3 = spmm_block(b, ag3_out[:], idx23_t, C3, C, CHUNKS3,
                                  CHMAX3, mp3, aps, close=not use_bias)
                if use_bias:
                    nc.tensor.matmul(agg3[:], ones_t,
                                     brow_t[:, 2 * D:2 * D + C],
                                     start=False, stop=True)
                o_sb = wp.tile([128, C], f32, tag="o")
                nc.scalar.activation(o_sb[:], agg3[:], COPY)
                nc.sync.dma_start(out_h[b * 128:(b + 1) * 128, :], o_sb[:])

    nc.compile()
    return nc


_CACHE = {}


def _get_prog(cfg, kt_blk, use_bias):
    key = (cfg["N"], cfg["D"], kt_blk, use_bias)
    if key not in _CACHE:
        _CACHE[key] = _build(cfg, kt_blk, use_bias)
    return _CACHE[key]


# ---------------------------------------------------------------- entry point
CFG_FULL = dict(N=10000, E=160000, D=1024, C=64, NBLK=10, KT_MIN=4)


def make_in_maps(ins, pp, cfg=None):
    """Per-core input maps (all device tensors fp16)."""
    cfg = cfg or CFG_FULL
    D, C, KD = cfg["D"], cfg["C"], cfg["D"] // 128
    w12 = np.stack([
        np.asarray(ins["W1"], np.float32).reshape(KD, 128, D).transpose(1, 0, 2),
        np.asarray(ins["W2"], np.float32).reshape(KD, 128, D).transpose(1, 0, 2),
    ]).astype(np.float16)
    w3 = (np.asarray(ins["W3"], np.float32).reshape(KD, 128, C)
          .transpose(1, 0, 2).astype(np.float16))
    biases = np.concatenate([
        np.asarray(ins["b1"], np.float32), np.asarray(ins["b2"], np.float32),
        np.asarray(ins["b3"], np.float32), np.ones(128, np.float32),
    ]).astype(np.float16)[None, :]
    ident = np.eye(128, dtype=np.float16)
    hx = np.asarray(ins["h"], np.float32).astype(np.float16)
    return [
        dict(hx=hx, sker=np.ascontiguousarray(pp["S"][c]),
             idx1=pp["idx1"][c], idx23=pp["idx23"][c],
             w12=w12, w3=w3, ident=ident, biases=biases)
        for c in range(N_CORES)
    ]


def kernel(h, src, dst, W1, b1, W2, b2, W3, b3, cfg=CFG_FULL):
    from concourse.bass_utils import run_bass_kernel_spmd

    h = np.asarray(h, np.float32)
    src = np.asarray(src, np.int32)
    dst = np.asarray(dst, np.int32)
    N, C = cfg["N"], cfg["C"]

    pp = _prep(h, src, dst, cfg)
    use_bias = bool(np.any(b1) or np.any(b2) or np.any(b3))
    nc = _get_prog(cfg, pp["kt_blk"], use_bias)

    ins = dict(h=h, W1=W1, b1=b1, W2=W2, b2=b2, W3=W3, b3=b3)
    in_maps = make_in_maps(ins, pp, cfg)
    res = run_bass_kernel_spmd(nc, in_maps, core_ids=list(range(N_CORES)))

    out = np.zeros((N, C), np.float32)
    rows = pp["row_of_node"]
    allout = np.concatenate([res.results[c]["out"] for c in range(N_CORES)],
                            axis=0)
    out[:, :] = allout[rows]
    return out


# revision 10
# speedup vs baseline: 1.5723x; 1.5723x over previous
"""3-layer GCN (DGL GraphConv, norm='both') on 8 Trainium2 NeuronCores.

Strategy (v2):
  - Nodes are packed into 80 balanced bins (128 slots each) by in-degree
    (greedy least-loaded), 10 bins per core -> 1280 padded rows/core.
  - Edges live with the owner (bin) of their dst node. segment_sum runs as
    "scatter matmuls" on the TensorEngine: for each dst block,
    agg[128d, D] += S_kt[128s, 128d].T @ msg_kt[128s, D]. msg rows are the
    DISTINCT src nodes of the block's edges (deduped, ordered by gathered-id
    so chunks touch contiguous AllGather slabs); S carries
    sum-of-edge-weights norm_src[src]*norm_dst[dst] at (src_slot, dst_slot).
  - Layer 1 does NOT gather: the host pre-packs each core's msg rows
    (h[dedup src]) into a per-core contiguous table, streamed with plain
    HWDGE DMAs. Only layer 2 uses the SWDGE dma_gather (whose Q7
    descriptor generation, ~8.5ns/row, is the scarce resource).
  - The on-device datapath is fp16 (PSUM accumulation fp32): halves gather
    HBM traffic, AllGather bytes, and S/W loads.
  - Dense W matmuls per dst block: PE-transpose agg -> aggT, then
    x = aggT.T @ W (+ bias via K=1 matmul), ReLU fused into PSUM->SBUF.
  - Layer-1 outputs are exchanged with staged ncfw AllGathers so cores can
    gather any source row for layer 2's SpMM.
  - Layer 3 is PUSH-style (no gather, no 3rd AllGather): during layer 2
    each core keeps y3_j = x2_j @ W3 (64 wide) in SBUF; then
    partialT[64, dst] += y3_j.T-stationary @ S'_j (block-dense push S'
    streamed from HBM), PE-transpose, and one ReduceScatter sums the
    per-core partials, landing each core's own 1280 output rows.
"""
import sys
sys.path.insert(0, '/opt/trn_rl_repo')
import numpy as np

N_CORES = 8
CHMAX = 5        # k-tiles per layer-2 gather chunk
GB = 8           # output blocks per layer-3 S' stream batch


def _ag_splits(nblk):
    """Block-index boundaries of the staged AllGather slabs."""
    if nblk <= 2:
        return [0, nblk]
    fr = [0, round(0.3 * nblk), round(0.6 * nblk), round(0.8 * nblk),
          nblk - 1, nblk]
    return sorted(set(b for b in fr if 0 <= b <= nblk))


def _chunks(kt, chmax):
    """Split kt k-tiles into near-equal chunks of size <= chmax."""
    n = -(-kt // chmax)
    base, rem = divmod(kt, n)
    sizes = [base + 1] * rem + [base] * (n - rem)
    out, o = [], 0
    for s in sizes:
        out.append((o, s))
        o += s
    return out


# ---------------------------------------------------------------- host prep
def _partition_nodes(deg_in, n_nodes, nbins):
    """Greedy balanced-edge binning: nodes (sorted by in-degree desc) go to
    the least-loaded bin with a free slot (capacity 128)."""
    import heapq
    order = np.argsort(-deg_in, kind="stable")
    heap = [(0, b) for b in range(nbins)]
    heapq.heapify(heap)
    bin_of = np.empty(n_nodes, np.int32)
    slot_of = np.empty(n_nodes, np.int32)
    count = np.zeros(nbins, np.int64)
    load = np.zeros(nbins, np.int64)
    for n in order:
        while True:
            l, b = heapq.heappop(heap)
            if count[b] < 128:
                break
            # full bin: drop from heap permanently
        bin_of[n] = b
        slot_of[n] = count[b]
        count[b] += 1
        load[b] += int(deg_in[n])
        heapq.heappush(heap, (l + int(deg_in[n]), b))
    return bin_of, slot_of, load


def _prep(h, src, dst, cfg):
    """Build per-core S tiles, gather indices, slot->node maps, and the
    layer-3 push matrices."""
    N, E, NBLK = cfg["N"], cfg["E"], cfg["NBLK"]
    nbins = N_CORES * NBLK
    deg_out = np.bincount(src, minlength=N)
    deg_in = np.bincount(dst, minlength=N)
    norm_src = np.clip(deg_out, 1, None).astype(np.float32) ** np.float32(-0.5)
    norm_dst = np.clip(deg_in, 1, None).astype(np.float32) ** np.float32(-0.5)
    w = (norm_src[src] * norm_dst[dst]).astype(np.float32)

    bin_of, slot_of, load = _partition_nodes(deg_in, N, nbins)

    # deal bins to cores snake-wise by load to balance core totals
    order = np.argsort(-load, kind="stable")
    core_of_bin = np.empty(nbins, np.int32)
    blk_of_bin = np.empty(nbins, np.int32)
    nextblk = [0] * N_CORES
    for i, b in enumerate(order):
        r = i // N_CORES
        c = (i % N_CORES) if r % 2 == 0 else (N_CORES - 1 - (i % N_CORES))
        core_of_bin[b] = c
        blk_of_bin[b] = nextblk[c]
        nextblk[c] += 1

    RPC = NBLK * 128
    row_of_node = (core_of_bin[bin_of] * RPC + blk_of_bin[bin_of] * 128
                   + slot_of).astype(np.int32)
    # gather-id layout after the staged slab AllGathers: slab q holds rows
    # [b_q, e_q) of every core, concatenated core-major at offset 8*b_q
    sp = np.array(_ag_splits(NBLK)) * 128
    _c = row_of_node // RPC
    _r = row_of_node % RPC
    _q = np.searchsorted(sp, _r, side="right") - 1
    gid_of_node = (N_CORES * sp[_q] + _c * (sp[_q + 1] - sp[_q])
                   + _r - sp[_q]).astype(np.int32)

    # group edges by dst bin; dedup src nodes per bin, ordered by gid
    ebin = bin_of[dst]
    eorder = np.argsort(ebin, kind="stable")
    counts = np.bincount(ebin, minlength=nbins)
    bounds = np.concatenate([[0], np.cumsum(counts)])
    uniqs = []
    for b in range(nbins):
        es = eorder[bounds[b]:bounds[b + 1]]
        u, inv = np.unique(src[es], return_inverse=True)
        gorder = np.argsort(gid_of_node[u], kind="stable")
        rank = np.empty(len(u), np.int64)
        rank[gorder] = np.arange(len(u))
        uniqs.append((es, u[gorder], rank[inv]))
    max_u = max((len(u) for _, u, _ in uniqs), default=1)
    kt_blk = max(cfg["KT_MIN"], -(-max_u // 128))
    kt_tot = NBLK * kt_blk

    nodes_slot = np.zeros((N_CORES, kt_tot * 128), np.int32)
    idx23 = np.zeros((N_CORES, kt_tot * 128), np.int16)
    S = np.zeros((N_CORES, 128, kt_tot, 128), np.float32)
    for b in range(nbins):
        es, u, inv = uniqs[b]
        c, blk = int(core_of_bin[b]), int(blk_of_bin[b])
        p = np.arange(len(u))
        gpos = blk * kt_blk * 128 + p
        nodes_slot[c, gpos] = u
        idx23[c, gpos] = gid_of_node[u].astype(np.int16)
        kt = blk * kt_blk + inv // 128
        np.add.at(S[c], (inv % 128, kt, slot_of[dst[es]]), w[es])
    S = S.astype(np.float16)

    # layer-3 push matrices: S'[c][gb, src_slot, j, gi, dst_slot] sums w over
    # edges with src in (core c, block j) and dst in output block gb*GB+gi
    NGB = nbins // GB
    srow = row_of_node[src]
    drow = row_of_node[dst]
    c_u = srow // RPC
    j_u = (srow // 128) % NBLK
    s_u = srow % 128
    ob_v = drow // 128
    d_v = drow % 128
    sp3 = []
    for c in range(N_CORES):
        m = c_u == c
        spc = np.zeros((NGB, 128, NBLK, GB, 128), np.float32)
        np.add.at(spc, (ob_v[m] // GB, s_u[m], j_u[m], ob_v[m] % GB, d_v[m]),
                  w[m])
        sp3.append(spc.astype(np.float16))

    def wrap(ix):  # -> [128, kt_tot*8] wrapped for the 8 Q7 cores
        return np.tile(ix.reshape(-1, 16).T, (8, 1)).copy()

    idx23_w = np.stack([wrap(idx23[c]) for c in range(N_CORES)])
    return dict(S=S, idx23=idx23_w, nodes_slot=nodes_slot, sp3=sp3,
                row_of_node=row_of_node, kt_blk=kt_blk, kt_tot=kt_tot)


# ---------------------------------------------------------------- device prog
def _build(cfg, kt_blk, use_bias):
    import concourse.bacc as bacc
    import concourse.mybir as mybir
    import concourse.tile as tile
    from concourse.library_config import mlp

    f32 = mybir.dt.float32
    f16 = mybir.dt.float16
    i16 = mybir.dt.int16
    RELU = mybir.ActivationFunctionType.Relu
    COPY = mybir.ActivationFunctionType.Copy

    N, D, C, NBLK = cfg["N"], cfg["D"], cfg["C"], cfg["NBLK"]
    RPC = NBLK * 128
    NPAD = N_CORES * RPC
    KT = kt_blk
    KT_TOT = NBLK * KT
    NGB = N_CORES * NBLK // GB      # layer-3 stream batches
    CHUNKS = _chunks(KT, CHMAX)
    KD = D // 128               # dense contraction k-tiles
    ND = 512 if D % 512 == 0 else D
    NT = D // ND                # dense n-tiles
    TPW = min(512, D)           # transposes packed per tps tile
    TPG = TPW // 128
    SPL = _ag_splits(NBLK)

    nc = bacc.Bacc("TRN2", target_bir_lowering=False, debug=False,
                   num_devices=N_CORES, num_swdge_queues=4,
                   dynamic_dma_scratch_size=32768)

    hxp = nc.dram_tensor("hxp", [KT_TOT * 128, D], f16, kind="ExternalInput")
    sker = nc.dram_tensor("sker", [128, KT_TOT, 128], f16, kind="ExternalInput")
    idx23_h = nc.dram_tensor("idx23", [128, KT_TOT * 8], i16, kind="ExternalInput")
    w12_h = nc.dram_tensor("w12", [2, 128, KD, D], f16, kind="ExternalInput")
    w3_h = nc.dram_tensor("w3", [128, KD, C], f16, kind="ExternalInput")
    ident_h = nc.dram_tensor("ident", [128, 128], f16, kind="ExternalInput")
    bias_h = nc.dram_tensor("biases", [1, 2 * D + C + 128], f16, kind="ExternalInput")
    b3c_h = nc.dram_tensor("b3c", [C, 1], f32, kind="ExternalInput")
    sp3_h = nc.dram_tensor("sp3", [NGB, 128, NBLK, GB, 128], f16,
                           kind="ExternalInput")
    out_h = nc.dram_tensor("out", [RPC, C], f32, kind="ExternalOutput")

    ag_in = nc.dram_tensor("ag_in", [RPC, D], f16, kind="Internal")
    ag_out = nc.dram_tensor("ag_out", [NPAD, D], f16, kind="Internal",
                            addr_space="Shared")
    part_h = nc.dram_tensor("part", [NPAD, C], f16, kind="Internal")
    rs_out = nc.dram_tensor("rs_out", [RPC, C], f16, kind="Internal")

    with tile.TileContext(nc) as tc:
        nc.gpsimd.load_library(mlp)
        with (
            tc.tile_pool(name="const", bufs=1) as cp,
            tc.tile_pool(name="work", bufs=2) as wp,
            tc.tile_pool(name="densps", bufs=2, space="PSUM") as dps,
            tc.tile_pool(name="tpsps", bufs=2, space="PSUM") as tps,
        ):
            s_blk = []
            for b in range(NBLK):
                sb = cp.tile([128, KT, 128], f16, tag=f"s{b}")
                nc.sync.dma_start(sb[:], sker[:, b * KT:(b + 1) * KT, :])
                s_blk.append(sb)
                if b == 0:
                    w_t = cp.tile([128, KD, D], f16, tag="w")
                    nc.sync.dma_start(w_t[:], w12_h[0])
                    ident_t = cp.tile([128, 128], f16, tag="ident")
                    nc.sync.dma_start(ident_t[:], ident_h[:])
            idx23_t = cp.tile([128, KT_TOT * 8], i16, tag="idx23")
            nc.sync.dma_start(idx23_t[:], idx23_h[:])
            w3_t = cp.tile([128, KD, C], f16, tag="w3")
            nc.sync.dma_start(w3_t[:], w3_h[:])
            if use_bias:
                brow_t = cp.tile([1, 2 * D + C + 128], f16, tag="brow")
                nc.sync.dma_start(brow_t[:], bias_h[:])
                ones_t = brow_t[:, 2 * D + C:2 * D + C + 128]
                b3c_t = cp.tile([C, 1], f32, tag="b3c")
                nc.sync.dma_start(b3c_t[:], b3c_h[:])
            y3_t = [cp.tile([128, C], f16, tag=f"y3{j}", name=f"y3_{j}")
                    for j in range(NBLK)]

            qctr = [0]

            def spmm_block(b, layer, msg_pool, psum_pool):
                """agg[128, D] for dst block b via msg fetch + scatter MMs.
                Layer 1 streams pre-packed rows (HWDGE); layer 2 gathers
                AllGathered rows with SWDGE dma_gather."""
                agg = psum_pool.tile([128, D], f32, tag="aggps")
                nspl = max(1, D // 512)
                for c0, ch in CHUNKS:
                    msg = msg_pool.tile([128, CHMAX, D], f16, tag="m")
                    if layer == 0:
                        r0 = (b * KT + c0) * 128
                        r1 = r0 + ch * 128
                        nc.sync.dma_start(
                            msg[:, :ch, :],
                            hxp[r0:r1].rearrange("(a p) d -> p a d", p=128))
                    else:
                        col0 = (b * KT + c0) * 8
                        q = qctr[0] % 4
                        qctr[0] += 1
                        nc.gpsimd.dma_gather(
                            msg[:, :ch, :], ag_out[:],
                            idx23_t[:, col0:col0 + ch * 8],
                            ch * 128, ch * 128, D, queue_num=q)
                    for k in range(ch):
                        kt = c0 + k
                        first = (c0 == 0 and k == 0)
                        last = (c0 + ch == KT and k == ch - 1)
                        for n in range(nspl):
                            w0 = n * (D // nspl)
                            w1 = (n + 1) * (D // nspl)
                            nc.tensor.matmul(
                                agg[:, w0:w1], s_blk[b][:, kt, :],
                                msg[:, k, w0:w1],
                                start=first, stop=last)
                return agg

            def transpose_to(dst_t, src_sb):
                """dst_t[128, KD, 128] (f16) = src_sb[128, D] transposed."""
                for g in range(KD // TPG):
                    tp = tps.tile([128, TPW], f16, tag="tp")
                    for j in range(TPG):
                        col = (g * TPG + j) * 128
                        nc.tensor.transpose(
                            tp[:, j * 128:(j + 1) * 128],
                            src_sb[:, col:col + 128], ident_t[:])
                    nc.vector.tensor_copy(
                        dst_t[:, g * TPG:(g + 1) * TPG, :].rearrange(
                            "p a b -> p (a b)"), tp[:])

            def dense_block(aggT_t, out_sb, bias_off, relu):
                """out_sb[128, D] = act(aggT.T @ W + b)."""
                for n in range(NT):
                    dp = dps.tile([128, ND], f32, tag="dp")
                    for k in range(KD):
                        nc.tensor.matmul(
                            dp[:], aggT_t[:, k, :], w_t[:, k, n * ND:(n + 1) * ND],
                            start=(k == 0), stop=(k == KD - 1 and not use_bias))
                    if use_bias:
                        nc.tensor.matmul(
                            dp[:], ones_t,
                            brow_t[:, bias_off + n * ND:bias_off + (n + 1) * ND],
                            start=False, stop=True)
                    nc.scalar.activation(out_sb[:, n * ND:(n + 1) * ND], dp[:],
                                         RELU if relu else COPY)

            # ---------------- layer 1 + 2
            with (
                tc.tile_pool(name="msg", bufs=3) as mp,
                tc.tile_pool(name="aggps", bufs=2, space="PSUM") as aps,
            ):
                for layer in range(2):
                    for b in range(NBLK):
                        agg = spmm_block(b, layer, mp, aps)
                        agg_sb = wp.tile([128, D], f16, tag="aggsb")
                        nc.scalar.activation(agg_sb[:], agg[:], COPY)
                        aggT_t = wp.tile([128, KD, 128], f16, tag="aggT")
                        transpose_to(aggT_t, agg_sb)
                        x_sb = wp.tile([128, D], f16, tag="x")
                        dense_block(aggT_t, x_sb, layer * D, relu=True)
                        if layer == 0:
                            nc.sync.dma_start(ag_in[b * 128:(b + 1) * 128, :],
                                              x_sb[:])
                            if b + 1 in SPL[1:]:
                                r0 = SPL[SPL.index(b + 1) - 1] * 128
                                r1 = (b + 1) * 128
                                nc.gpsimd.collective_compute(
                                    "AllGather", mybir.AluOpType.bypass,
                                    ins=[ag_in[r0:r1, :]],
                                    outs=[ag_out[N_CORES * r0:N_CORES * r1, :]],
                                    replica_groups=[list(range(N_CORES))])
                        else:
                            # y3_b = x2_b @ W3, kept in SBUF for layer 3
                            x3T_t = wp.tile([128, KD, 128], f16, tag="x3T")
                            transpose_to(x3T_t, x_sb)
                            yp = dps.tile([128, C], f32, tag="dp")
                            for k in range(KD):
                                nc.tensor.matmul(yp[:], x3T_t[:, k, :],
                                                 w3_t[:, k, :],
                                                 start=(k == 0), stop=(k == KD - 1))
                            nc.scalar.activation(y3_t[b][:], yp[:], COPY)
                    if layer == 0:
                        nc.sync.dma_start(w_t[:], w12_h[1])

            # ---------------- layer 3 (push): partT = sum_j y3_j.T @ S'_j
            with (
                tc.tile_pool(name="spstream", bufs=2) as spp,
                tc.tile_pool(name="l3ps", bufs=2, space="PSUM") as lps,
            ):
                for gb in range(NGB):
                    spt = spp.tile([128, NBLK, GB, 128], f16, tag="sp")
                    nc.sync.dma_start(spt[:], sp3_h[gb])
                    pst = lps.tile([64, GB * 128], f32, tag="pst")
                    half = GB * 128 // 2
                    for j in range(NBLK):
                        for n in range(2):
                            nc.tensor.matmul(
                                pst[:, n * half:(n + 1) * half],
                                y3_t[j][:, :],
                                spt[:, j, :, :].rearrange(
                                    "p a b -> p (a b)")[:, n * half:(n + 1) * half],
                                start=(j == 0), stop=(j == NBLK - 1))
                    ps_sb = wp.tile([64, GB * 128], f16, tag="pssb")
                    if use_bias:
                        # add b3/8 on every core; the ReduceScatter sums to b3
                        nc.scalar.activation(ps_sb[:], pst[:], COPY,
                                             bias=b3c_t[:])
                    else:
                        nc.scalar.activation(ps_sb[:], pst[:], COPY)
                    for gi in range(GB):
                        tp = tps.tile([128, TPW], f16, tag="tp")
                        nc.tensor.transpose(tp[:, :64],
                                            ps_sb[:, gi * 128:(gi + 1) * 128],
                                            ident_t[:64, :64])
                        ob_sb = wp.tile([128, C], f16, tag="pt")
                        nc.vector.tensor_copy(ob_sb[:], tp[:, :64])
                        ob = gb * GB + gi
                        nc.sync.dma_start(part_h[ob * 128:(ob + 1) * 128, :],
                                          ob_sb[:])
                nc.gpsimd.collective_compute(
                    "ReduceScatter", mybir.AluOpType.add,
                    ins=[part_h[:]], outs=[rs_out[:]],
                    replica_groups=[list(range(N_CORES))])
                # rs_out (fp16, own rows) -> out (fp32)
                for b in range(NBLK):
                    t16 = wp.tile([128, C], f16, tag="of16")
                    nc.sync.dma_start(t16[:], rs_out[b * 128:(b + 1) * 128, :])
                    o_sb = wp.tile([128, C], f32, tag="o")
                    nc.vector.tensor_copy(o_sb[:], t16[:])
                    nc.sync.dma_start(out_h[b * 128:(b + 1) * 128, :], o_sb[:])

    nc.compile()
    return nc


_CACHE = {}


def _get_prog(cfg, kt_blk, use_bias):
    key = (cfg["N"], cfg["D"], kt_blk, use_bias)
    if key not in _CACHE:
        _CACHE[key] = _build(cfg, kt_blk, use_bias)
    return _CACHE[key]


# ---------------------------------------------------------------- entry point
CFG_FULL = dict(N=10000, E=160000, D=1024, C=64, NBLK=10, KT_MIN=4)


def make_in_maps(ins, pp, cfg=None):
    """Per-core input maps (all device tensors fp16)."""
    cfg = cfg or CFG_FULL
    D, C = cfg["D"], cfg["C"]
    KD = D // 128
    w12 = np.stack([
        np.asarray(ins["W1"], np.float32).reshape(KD, 128, D).transpose(1, 0, 2),
        np.asarray(ins["W2"], np.float32).reshape(KD, 128, D).transpose(1, 0, 2),
    ]).astype(np.float16)
    w3 = (np.asarray(ins["W3"], np.float32).reshape(KD, 128, C)
          .transpose(1, 0, 2).astype(np.float16))
    biases = np.concatenate([
        np.asarray(ins["b1"], np.float32), np.asarray(ins["b2"], np.float32),
        np.asarray(ins["b3"], np.float32), np.ones(128, np.float32),
    ]).astype(np.float16)[None, :]
    ident = np.eye(128, dtype=np.float16)
    b3c = (np.asarray(ins["b3"], np.float32) / N_CORES).reshape(C, 1)
    h16 = np.asarray(ins["h"], np.float32).astype(np.float16)
    return [
        dict(hxp=np.ascontiguousarray(h16[pp["nodes_slot"][c]]),
             sker=np.ascontiguousarray(pp["S"][c]),
             idx23=pp["idx23"][c], sp3=pp["sp3"][c],
             w12=w12, w3=w3, ident=ident, biases=biases, b3c=b3c)
        for c in range(N_CORES)
    ]


def kernel(h, src, dst, W1, b1, W2, b2, W3, b3, cfg=CFG_FULL):
    from concourse.bass_utils import run_bass_kernel_spmd

    h = np.asarray(h, np.float32)
    src = np.asarray(src, np.int32)
    dst = np.asarray(dst, np.int32)
    N, C = cfg["N"], cfg["C"]

    pp = _prep(h, src, dst, cfg)
    use_bias = bool(np.any(b1) or np.any(b2) or np.any(b3))
    nc = _get_prog(cfg, pp["kt_blk"], use_bias)

    ins = dict(h=h, W1=W1, b1=b1, W2=W2, b2=b2, W3=W3, b3=b3)
    in_maps = make_in_maps(ins, pp, cfg)
    res = run_bass_kernel_spmd(nc, in_maps, core_ids=list(range(N_CORES)))

    out = np.zeros((N, C), np.float32)
    rows = pp["row_of_node"]
    allout = np.concatenate([res.results[c]["out"] for c in range(N_CORES)],
                            axis=0)
    out[:, :] = allout[rows]
    return out
